# revision 18
# baseline (speedup 1.0000x reference)
"""Trainium2 Bass kernel for nn_Embedding2Score (session-graph attention +
vocab-scored readout).

Sharding (8 NeuronCores):
  - phase 1 (attention + segment pooling): data-parallel over sessions —
    core k owns graphs [k*128, (k+1)*128) == nodes [k*6400, (k+1)*6400).
  - each core folds its pooled s_g into its own s_h^T block (tiny matmul),
    then one AllGather exchanges the 8 s_h^T blocks (32KB/core). Gathered
    blocks are DMA'd straight into the s_h^T tile — they are exactly the
    lhsT operands for phase 2, so remote blocks need zero post-collective
    compute besides the z matmuls themselves.
  - phase 2 (z = s_h @ item_emb.T): tensor-parallel over the vocab V —
    core k owns item columns [k*12500, (k+1)*12500) and emits z[:, shard].

All matrices are kept in "transposed" (feature-on-partition) layout on
device so every matmul uses the natural [in,out] weight storage as lhsT
with zero on-device transposes. Segment broadcast (v_n -> nodes) and
segment sum are matmuls against 0/1 selector matrices E2 ([graph, node])
and its transpose — constants for the uniform L=50 layout.

Matmul operands are bf16 (PSUM accumulation stays f32). z is written to
DRAM as bf16 and cast to f32 on the host: the store stream is the
roofline term (B*V elements), so halving it halves phase-2 wall time,
and bf16 rounding (~2^-9 relative) is far inside the accuracy budget.

Latency hiding: the collectives runtime pays a one-time bootstrap
barrier that absorbs the inter-core launch skew; each core computes its
OWN graph block's z first (purely local) to overlap that window, and
the remaining 7 blocks' s_h arrive via rank-rotated (partition-id
offset) reads of the gathered buffer; the host un-rotates the z row
blocks. Loads are few, large, and ordered critical-first across both
HWDGE rings because each ring executes its DMAs serially.
"""

from contextlib import ExitStack

import numpy as np

H = 128
B = 1024
L = 50
N = B * L
V = 100000
M = 8            # cores
Bs = B // M      # 128 graphs / core
Ns = N // M      # 6400 nodes / core
Vs = V // M      # 12500 vocab cols / core
NT = Ns // H     # 50 node tiles / core
CH = 512         # phase-1a chunk width (nodes)
ZCH = 512        # phase-2 psum chunk width (vocab cols, 1 PSUM bank)


def _sigmoid(x):
    out = np.empty_like(x)
    np.negative(x, out=out)
    np.exp(out, out=out)
    out += 1.0
    np.reciprocal(out, out=out)
    return out


def _kernel_numpy(session, item, batch, W1, b1, W2, b2, q, bq, W3, b3):
    """General-batch fallback (host only). Handles any sorted batch."""
    nb = int(batch.max()) + 1
    last_idx = np.searchsorted(batch, np.arange(nb), side="right") - 1
    v_n = session[last_idx]
    pre = _sigmoid(v_n[batch] @ W1 + b1 + session @ W2 + b2)
    alpha = pre @ q + bq
    w = alpha * session
    s_g = np.zeros((nb, session.shape[1]), np.float32)
    np.add.at(s_g, batch, w)
    s_h = np.concatenate([v_n, s_g], axis=1) @ W3 + b3
    return (s_h @ item.T).astype(np.float32)


def _build_program(bq_val):
    import concourse.bass as bass
    import concourse.bacc as bacc
    import concourse.tile as tile
    from concourse import mybir

    F32 = mybir.dt.float32
    BF16 = mybir.dt.bfloat16
    SIG = mybir.ActivationFunctionType.Sigmoid
    IDN = mybir.ActivationFunctionType.Identity

    nc = bacc.Bacc("TRN2", target_bir_lowering=False, debug=False,
                   num_devices=M)

    # ---- DRAM I/O (per-core data; identical program on all cores) ----
    d_xT = nc.dram_tensor("xT", [H, Ns], BF16, kind="ExternalInput").ap()
    d_e2 = nc.dram_tensor("e2", [Bs, Ns], BF16, kind="ExternalInput").ap()
    d_xnm = nc.dram_tensor("xnm", [H, NT, H], BF16, kind="ExternalInput").ap()
    d_e2t = nc.dram_tensor("e2t", [H, NT, Bs], BF16,
                           kind="ExternalInput").ap()
    # own-shard v_n^T [H, Bs] ++ packed biases (bc | b3 as f32, bitcast into
    # 4 bf16 columns) so the sync ring needs one DMA for both.
    d_vno = nc.dram_tensor("vno", [H, Bs + 4], BF16,
                           kind="ExternalInput").ap()
    d_item = nc.dram_tensor("itemT", [H, Vs], BF16, kind="ExternalInput").ap()
    # packed weights: w1 | w2 | w3a | w3b | q  (columns), bf16
    d_wpk = nc.dram_tensor("wpk", [H, 4 * H + 1], BF16,
                           kind="ExternalInput").ap()
    # z rows are in LOCAL block order; the host maps local block j to
    # global graph block (rank+j)%M when assembling the full output.
    d_z = nc.dram_tensor("z", [B, Vs], BF16, kind="ExternalOutput").ap()

    cc_in = nc.dram_tensor("cc_in", [H, Bs], BF16).ap()
    cc_out = nc.dram_tensor("cc_out", [M * H, Bs], BF16,
                            addr_space="Shared").ap()

    with tile.TileContext(nc) as tc, ExitStack() as ctx:
        nc_ = tc.nc

        consts = ctx.enter_context(tc.tile_pool(name="consts", bufs=1))
        small = ctx.enter_context(tc.tile_pool(name="small", bufs=1))
        item_pool = ctx.enter_context(tc.tile_pool(name="itemp", bufs=1))
        work = ctx.enter_context(tc.tile_pool(name="work", bufs=3))
        big1 = ctx.enter_context(tc.tile_pool(name="big1", bufs=1))
        zout = ctx.enter_context(tc.tile_pool(name="zout", bufs=2))
        psum_a = ctx.enter_context(
            tc.tile_pool(name="psum_a", bufs=2, space="PSUM"))
        # p_alpha and p_sg share one bank (phases 1a/1c are sequential)
        psum_s = ctx.enter_context(
            tc.tile_pool(name="psum_s", bufs=1, space="PSUM"))
        psum_z = ctx.enter_context(
            tc.tile_pool(name="psum_z", bufs=5, space="PSUM"))

        # ---- inputs: 8 large DMAs, critical-first, split over the two
        # HWDGE rings (each ring runs its DMAs serially).
        wpk_sb = consts.tile([H, 4 * H + 1], BF16)
        vno_sb = consts.tile([H, Bs + 4], BF16)
        itemT_sb = item_pool.tile([H, Vs], BF16)
        xT_sb = big1.tile([H, Ns], BF16)
        e2_sb = big1.tile([Bs, Ns], BF16)
        xnm_sb = big1.tile([H, NT, H], BF16)
        e2t_sb = big1.tile([H, NT, Bs], BF16)

        vh = Vs // 2
        # scalar (ACT HWDGE) ring
        nc_.scalar.dma_start(out=wpk_sb[:], in_=d_wpk[:])
        nc_.scalar.dma_start(out=xT_sb[:], in_=d_xT[:])
        nc_.scalar.dma_start(out=xnm_sb[:], in_=d_xnm[:])
        nc_.scalar.dma_start(out=itemT_sb[:, :vh], in_=d_item[:, :vh])
        # sync (SP HWDGE) ring
        nc_.sync.dma_start(out=vno_sb[:], in_=d_vno[:])
        nc_.sync.dma_start(out=e2_sb[:], in_=d_e2[:])
        nc_.sync.dma_start(out=e2t_sb[:], in_=d_e2t[:])
        nc_.sync.dma_start(out=itemT_sb[:, vh:], in_=d_item[:, vh:])

        w1s = wpk_sb[:, 0 * H:1 * H]
        w2s = wpk_sb[:, 1 * H:2 * H]
        w3as = wpk_sb[:, 2 * H:3 * H]
        w3bs = wpk_sb[:, 3 * H:4 * H]
        qs = wpk_sb[:, 4 * H:4 * H + 1]
        bpk = vno_sb[:, Bs:Bs + 4].bitcast(F32)
        bcs = bpk[:, 0:1]
        b3s = bpk[:, 1:2]

        # misc phase-1 results that outlive their producers
        av_sb = small.tile([H, H], BF16)       # (v_n @ W1), graph-major
        alpha_sb = small.tile([H, NT], F32)    # node-tile columns of alpha
        sg_sb = small.tile([H, Bs], BF16)      # s_g^T local shard
        shT_sb = small.tile([H, B], BF16)      # s_h^T, local block order

        # Av = v_n_shard @ W1   -> [graph, h_out]
        p_av = psum_a.tile([H, CH], F32, tag="pp", name="p_av")
        nc_.tensor.matmul(p_av[:, :H], lhsT=vno_sb[:, :Bs], rhs=w1s,
                          start=True, stop=True)
        nc_.scalar.copy(out=av_sb[:], in_=p_av[:, :H])

        # phase 1a: S^T = sigmoid(W2^T X^T + Av^T E2 + bc) ; alpha columns
        p_small = psum_s.tile([H, 192], F32)
        p_alpha = p_small[:, 0:NT]
        n_chunks = (Ns + CH - 1) // CH
        for c in range(n_chunks):
            c0 = c * CH
            cw = min(CH, Ns - c0)
            pp = psum_a.tile([H, CH], F32, tag="pp")
            nc_.tensor.matmul(pp[:, :cw], lhsT=w2s,
                              rhs=xT_sb[:, c0:c0 + cw],
                              start=True, stop=False)
            nc_.tensor.matmul(pp[:, :cw], lhsT=av_sb[:],
                              rhs=e2_sb[:, c0:c0 + cw],
                              start=False, stop=True)
            s_sb = work.tile([H, CH], BF16, tag="schunk")
            nc_.scalar.activation(s_sb[:, :cw], pp[:, :cw], SIG, bias=bcs)
            for s in range(cw // H):
                t = c * (CH // H) + s
                nc_.tensor.matmul(p_alpha[:, t:t + 1],
                                  lhsT=s_sb[:, s * H:(s + 1) * H],
                                  rhs=qs, start=True, stop=True)
        # alpha = (S^T)^T q + bq, one column per node tile
        nc_.vector.tensor_scalar_add(alpha_sb[:], p_alpha, float(bq_val))

        # phase 1c: s_g^T = sum_t (X_t * alpha_t)^T E2T_t
        p_sg = p_small[:, 64:64 + Bs]
        for t in range(NT):
            xa = work.tile([H, H], BF16, tag="xa")
            nc_.vector.tensor_scalar_mul(xa[:], xnm_sb[:, t, :],
                                         alpha_sb[:, t:t + 1])
            nc_.tensor.matmul(p_sg, lhsT=xa[:], rhs=e2t_sb[:, t, :],
                              start=(t == 0), stop=(t == NT - 1))
        nc_.vector.tensor_copy(out=sg_sb[:], in_=p_sg)

        # own s_h^T block: W3a^T v_n + W3b^T s_g + b3
        p_sh = psum_a.tile([H, CH], F32, tag="pp", name="p_sh")
        nc_.tensor.matmul(p_sh[:, :Bs], lhsT=w3as, rhs=vno_sb[:, :Bs],
                          start=True, stop=False)
        nc_.tensor.matmul(p_sh[:, :Bs], lhsT=w3bs, rhs=sg_sb[:],
                          start=False, stop=True)
        nc_.scalar.activation(shT_sb[:, :Bs], p_sh[:, :Bs], IDN, bias=b3s)

        # collective: gather every shard's s_h^T (block r = rank r's s_h)
        nc_.scalar.dma_start(out=cc_in[:], in_=shT_sb[:, :Bs])
        nc_.gpsimd.collective_compute(
            "AllGather", mybir.AluOpType.bypass,
            replica_groups=[list(range(M))],
            ins=[cc_in.opt()], outs=[cc_out.opt()])

        # gathered s_h blocks land straight in shT_sb at rank-rotated
        # offsets; alternating sync/gpsimd queues drain them in parallel
        # the moment the collective completes.
        rank_g = nc_.gpsimd.partition_id()
        rank_s = nc_.sync.partition_id()
        for j in range(1, M):
            if j % 2 == 0:
                eng, rank = nc_.gpsimd, rank_g
            else:
                eng, rank = nc_.sync, rank_s
            src0 = ((rank + j) % M) * H
            eng.dma_start(out=shT_sb[:, j * H:(j + 1) * H],
                          in_=cc_out[bass.ds(src0, H), :])

        eng_i = 0

        # z writes: 4 pieces per block, each issued right after the drains
        # covering its columns so the store stream flows during the block
        # (and the final block's flush is only ~1/4 of its bytes).
        PIECE = 6 * ZCH                       # 3072 cols; last piece 3284

        def z_block(bci):
            nonlocal eng_i
            lhs = shT_sb[:, bci * H:(bci + 1) * H]
            zt = zout.tile([H, Vs], BF16, tag="zt")
            wi = 0
            for u in range(0, Vs, ZCH):
                uw = min(ZCH, Vs - u)
                zp = psum_z.tile([H, ZCH], F32, tag="zp")
                nc_.tensor.matmul(zp[:, :uw], lhsT=lhs,
                                  rhs=itemT_sb[:, u:u + uw],
                                  start=True, stop=True)
                if eng_i % 9 < 5:
                    nc_.vector.tensor_copy(out=zt[:, u:u + uw],
                                           in_=zp[:, :uw])
                else:
                    nc_.scalar.copy(out=zt[:, u:u + uw], in_=zp[:, :uw])
                eng_i += 1
                done = u + uw
                if wi < 3 and done == (wi + 1) * PIECE:
                    ring = nc_.sync if (bci + wi) % 2 == 0 else nc_.scalar
                    ring.dma_start(
                        out=d_z[bci * H:(bci + 1) * H, wi * PIECE:done],
                        in_=zt[:, wi * PIECE:done])
                    wi += 1
            ring = nc_.sync if (bci + 3) % 2 == 0 else nc_.scalar
            ring.dma_start(out=d_z[bci * H:(bci + 1) * H, 3 * PIECE:],
                           in_=zt[:, 3 * PIECE:])

        for bci in range(M):
            z_block(bci)

    nc.compile()
    return nc


_CACHE = {}


def _get_program(bq_val):
    key = round(float(bq_val), 10)
    if key not in _CACHE:
        _CACHE[key] = _build_program(bq_val)
    return _CACHE[key]


def kernel(session_embedding, item_emb, batch, num_graphs,
           W1, b1, W2, b2, q, bq, W3, b3):
    import ml_dtypes
    BF = ml_dtypes.bfloat16

    session = np.ascontiguousarray(np.asarray(session_embedding, np.float32))
    item = np.ascontiguousarray(np.asarray(item_emb, np.float32))
    batch = np.asarray(batch)
    W1 = np.asarray(W1, np.float32)
    b1 = np.asarray(b1, np.float32)
    W2 = np.asarray(W2, np.float32)
    b2 = np.asarray(b2, np.float32)
    q = np.asarray(q, np.float32)
    bq = np.asarray(bq, np.float32)
    W3 = np.asarray(W3, np.float32)
    b3 = np.asarray(b3, np.float32)

    uniform = (session.shape == (N, H) and item.shape == (V, H)
               and batch.shape == (N,)
               and int(num_graphs) == B
               and np.array_equal(batch, np.repeat(np.arange(B), L)))
    if not uniform:
        return _kernel_numpy(session, item, batch, W1, b1, W2, b2,
                             q, bq, W3, b3)

    from concourse.bass_utils import run_bass_kernel_spmd

    nc = _get_program(bq[0])

    # ---- host-side shard prep (index bookkeeping + bf16 casts) ----
    last_idx = np.arange(B) * L + (L - 1)
    v_n = session[last_idx]                       # [B, H]
    vnfT = np.ascontiguousarray(v_n.T.astype(BF))  # [H, B]

    gidx = (np.arange(Ns) // L).astype(np.int64)
    E2 = np.zeros((Bs, Ns), BF)
    E2[gidx, np.arange(Ns)] = 1.0
    E2T_t = np.ascontiguousarray(
        E2.T.reshape(NT, H, Bs).transpose(1, 0, 2))  # [H, NT, Bs]

    itemT = np.ascontiguousarray(item.T.astype(BF))  # [H, V]
    sessT = session.T.astype(BF)                     # [H, N]

    wpk = np.empty((H, 4 * H + 1), BF)
    wpk[:, 0 * H:1 * H] = W1.astype(BF)
    wpk[:, 1 * H:2 * H] = W2.astype(BF)
    wpk[:, 2 * H:3 * H] = W3[:H].astype(BF)
    wpk[:, 3 * H:4 * H] = W3[H:].astype(BF)
    wpk[:, 4 * H] = q.reshape(H).astype(BF)
    bpk = np.empty((H, 2), np.float32)
    bpk[:, 0] = b1 + b2
    bpk[:, 1] = b3
    bpk_bits = bpk.view(np.uint16).view(BF)          # [H, 4] raw bf16 view

    in_maps = []
    for k in range(M):
        nsl = slice(k * Ns, (k + 1) * Ns)
        xnm_t = np.ascontiguousarray(
            session[nsl].astype(BF).reshape(NT, H, H)
            .transpose(1, 0, 2))                     # [H, NT, H]
        vno = np.empty((H, Bs + 4), BF)
        vno[:, :Bs] = vnfT[:, k * Bs:(k + 1) * Bs]
        vno[:, Bs:] = bpk_bits
        in_maps.append({
            "xT": np.ascontiguousarray(sessT[:, nsl]),
            "e2": E2,
            "xnm": xnm_t,
            "e2t": E2T_t,
            "vno": vno,
            "itemT": np.ascontiguousarray(itemT[:, k * Vs:(k + 1) * Vs]),
            "wpk": wpk,
        })

    res = run_bass_kernel_spmd(nc, in_maps, list(range(M)))

    # un-rotate: core k's local z row-block j holds graphs ((k+j)%M)*Bs..
    z = np.empty((B, V), np.float32)
    for k in range(M):
        zk = np.asarray(res.results[k]["z"]).astype(np.float32)
        for j in range(M):
            gblk = (k + j) % M
            z[gblk * Bs:(gblk + 1) * Bs, k * Vs:(k + 1) * Vs] = \
                zk[j * Bs:(j + 1) * Bs]
    return z


# revision 23
# speedup vs baseline: 1.0649x; 1.0649x over previous
"""Trainium2 Bass kernel for nn_Embedding2Score (session-graph attention +
vocab-scored readout).

Sharding (8 NeuronCores):
  - phase 1 (attention + segment pooling): data-parallel over sessions —
    core k owns graphs [k*128, (k+1)*128) == nodes [k*6400, (k+1)*6400).
  - each core folds its pooled s_g into its own s_h^T block (tiny matmul),
    then one AllGather exchanges the 8 s_h^T blocks (32KB/core). Gathered
    blocks are DMA'd straight into the s_h^T tile — they are exactly the
    lhsT operands for phase 2, so remote blocks need zero post-collective
    compute besides the z matmuls themselves.
  - phase 2 (z = s_h @ item_emb.T): tensor-parallel over the vocab V —
    core k owns item columns [k*12500, (k+1)*12500) and emits z[:, shard].

All matrices are kept in "transposed" (feature-on-partition) layout on
device so every matmul uses the natural [in,out] weight storage as lhsT
with zero on-device transposes. Segment broadcast (v_n -> nodes) and
segment sum are matmuls against 0/1 selector matrices E2 ([graph, node])
and its transpose — constants for the uniform L=50 layout.

Matmul operands are bf16 (PSUM accumulation stays f32). z is written to
DRAM as bf16 and cast to f32 on the host: the store stream is the
roofline term (B*V elements), so halving it halves phase-2 wall time,
and bf16 rounding (~2^-9 relative) is far inside the accuracy budget.

Latency hiding: the collectives runtime pays a one-time bootstrap
barrier that absorbs the inter-core launch skew; each core computes its
OWN graph block's z first (purely local) to overlap that window, and
the remaining 7 blocks' s_h arrive via rank-rotated (partition-id
offset) reads of the gathered buffer; the host un-rotates the z row
blocks. Loads are few, large, and ordered critical-first across both
HWDGE rings because each ring executes its DMAs serially.
"""

from contextlib import ExitStack

import numpy as np

H = 128
B = 1024
L = 50
N = B * L
V = 100000
M = 8            # cores
Bs = B // M      # 128 graphs / core
Ns = N // M      # 6400 nodes / core
Vs = V // M      # 12500 vocab cols / core
NT = Ns // H     # 50 node tiles / core
CH = 512         # phase-1a chunk width (nodes)
ZCH = 512        # phase-2 psum chunk width (vocab cols, 1 PSUM bank)


def _sigmoid(x):
    out = np.empty_like(x)
    np.negative(x, out=out)
    np.exp(out, out=out)
    out += 1.0
    np.reciprocal(out, out=out)
    return out


def _kernel_numpy(session, item, batch, W1, b1, W2, b2, q, bq, W3, b3):
    """General-batch fallback (host only). Handles any sorted batch."""
    nb = int(batch.max()) + 1
    last_idx = np.searchsorted(batch, np.arange(nb), side="right") - 1
    v_n = session[last_idx]
    pre = _sigmoid(v_n[batch] @ W1 + b1 + session @ W2 + b2)
    alpha = pre @ q + bq
    w = alpha * session
    s_g = np.zeros((nb, session.shape[1]), np.float32)
    np.add.at(s_g, batch, w)
    s_h = np.concatenate([v_n, s_g], axis=1) @ W3 + b3
    return (s_h @ item.T).astype(np.float32)


def _build_program(bq_val):
    import concourse.bass as bass
    import concourse.bacc as bacc
    import concourse.tile as tile
    from concourse import mybir

    F32 = mybir.dt.float32
    BF16 = mybir.dt.bfloat16
    SIG = mybir.ActivationFunctionType.Sigmoid
    IDN = mybir.ActivationFunctionType.Identity

    nc = bacc.Bacc("TRN2", target_bir_lowering=False, debug=False,
                   num_devices=M)

    # ---- DRAM I/O (per-core data; identical program on all cores) ----
    d_xT = nc.dram_tensor("xT", [H, Ns], BF16, kind="ExternalInput").ap()
    d_e2 = nc.dram_tensor("e2", [Bs, Ns], BF16, kind="ExternalInput").ap()
    d_xnm = nc.dram_tensor("xnm", [H, NT, H], BF16, kind="ExternalInput").ap()
    d_e2t = nc.dram_tensor("e2t", [H, NT, Bs], BF16,
                           kind="ExternalInput").ap()
    # neighbor shard (rank+1)%M node data: its phase 1 is recomputed
    # locally during the collective window so block 1 needs no gather.
    d_xT2 = nc.dram_tensor("xT2", [H, Ns], BF16, kind="ExternalInput").ap()
    d_xnm2 = nc.dram_tensor("xnm2", [H, NT, H], BF16,
                            kind="ExternalInput").ap()
    # own + neighbor v_n^T [H, 2*Bs] ++ packed biases (bc | b3 as f32,
    # bitcast into 4 bf16 columns) so the sync ring needs one DMA for all.
    d_vno = nc.dram_tensor("vno", [H, 2 * Bs + 4], BF16,
                           kind="ExternalInput").ap()
    d_item = nc.dram_tensor("itemT", [H, Vs], BF16, kind="ExternalInput").ap()
    # packed weights: w1 | w2 | w3a | w3b | q  (columns), bf16
    d_wpk = nc.dram_tensor("wpk", [H, 4 * H + 1], BF16,
                           kind="ExternalInput").ap()
    # z rows are in LOCAL block order; the host maps local block j to
    # global graph block (rank+j)%M when assembling the full output.
    d_z = nc.dram_tensor("z", [B, Vs], BF16, kind="ExternalOutput").ap()

    cc_in = nc.dram_tensor("cc_in", [H, Bs], BF16).ap()
    cc_out = nc.dram_tensor("cc_out", [M * H, Bs], BF16,
                            addr_space="Shared").ap()

    with tile.TileContext(nc) as tc, ExitStack() as ctx:
        nc_ = tc.nc

        consts = ctx.enter_context(tc.tile_pool(name="consts", bufs=1))
        small = ctx.enter_context(tc.tile_pool(name="small", bufs=1))
        item_pool = ctx.enter_context(tc.tile_pool(name="itemp", bufs=1))
        work = ctx.enter_context(tc.tile_pool(name="work", bufs=3))
        big1 = ctx.enter_context(tc.tile_pool(name="big1", bufs=1))
        zout = ctx.enter_context(tc.tile_pool(name="zout", bufs=2))
        psum_a = ctx.enter_context(
            tc.tile_pool(name="psum_a", bufs=2, space="PSUM"))
        # p_alpha and p_sg share one bank (phases 1a/1c are sequential)
        psum_s = ctx.enter_context(
            tc.tile_pool(name="psum_s", bufs=1, space="PSUM"))
        psum_z = ctx.enter_context(
            tc.tile_pool(name="psum_z", bufs=5, space="PSUM"))

        # ---- inputs: 8 large DMAs, critical-first, split over the two
        # HWDGE rings (each ring runs its DMAs serially).
        wpk_sb = consts.tile([H, 4 * H + 1], BF16)
        vno_sb = consts.tile([H, 2 * Bs + 4], BF16)
        itemT_sb = item_pool.tile([H, Vs], BF16)
        xT_sb = big1.tile([H, Ns], BF16)
        e2_sb = big1.tile([Bs, Ns], BF16)
        xnm_sb = big1.tile([H, NT, H], BF16)
        e2t_sb = big1.tile([H, NT, Bs], BF16)
        xT2_sb = big1.tile([H, Ns], BF16)
        xnm2_sb = big1.tile([H, NT, H], BF16)

        vh = Vs // 2
        # scalar (ACT HWDGE) ring
        nc_.scalar.dma_start(out=wpk_sb[:], in_=d_wpk[:])
        nc_.scalar.dma_start(out=xT_sb[:], in_=d_xT[:])
        nc_.scalar.dma_start(out=xnm_sb[:], in_=d_xnm[:])
        nc_.scalar.dma_start(out=xT2_sb[:], in_=d_xT2[:])
        nc_.scalar.dma_start(out=itemT_sb[:, :vh], in_=d_item[:, :vh])
        # sync (SP HWDGE) ring
        nc_.sync.dma_start(out=vno_sb[:], in_=d_vno[:])
        nc_.sync.dma_start(out=e2_sb[:], in_=d_e2[:])
        nc_.sync.dma_start(out=e2t_sb[:], in_=d_e2t[:])
        nc_.sync.dma_start(out=xnm2_sb[:], in_=d_xnm2[:])
        nc_.sync.dma_start(out=itemT_sb[:, vh:], in_=d_item[:, vh:])

        w1s = wpk_sb[:, 0 * H:1 * H]
        w2s = wpk_sb[:, 1 * H:2 * H]
        w3as = wpk_sb[:, 2 * H:3 * H]
        w3bs = wpk_sb[:, 3 * H:4 * H]
        qs = wpk_sb[:, 4 * H:4 * H + 1]
        bpk = vno_sb[:, 2 * Bs:2 * Bs + 4].bitcast(F32)
        bcs = bpk[:, 0:1]
        b3s = bpk[:, 1:2]

        # misc phase-1 results that outlive their producers
        av_sb = small.tile([H, H], BF16)       # (v_n @ W1), graph-major
        alpha_sb = small.tile([H, NT], F32)    # node-tile columns of alpha
        sg_sb = small.tile([H, Bs], BF16)      # s_g^T local shard
        shT_sb = small.tile([H, B], BF16)      # s_h^T, local block order

        p_small = psum_s.tile([H, 192], F32)
        p_alpha = p_small[:, 0:NT]
        p_sg = p_small[:, 64:64 + Bs]
        n_chunks = (Ns + CH - 1) // CH

        def phase1(xT_t, xnm_t, vn, sh_out):
            """attention + segment pooling + s_h fold for one graph shard"""
            # Av = v_n_shard @ W1   -> [graph, h_out]
            p_av = psum_a.tile([H, CH], F32, tag="pp")
            nc_.tensor.matmul(p_av[:, :H], lhsT=vn, rhs=w1s,
                              start=True, stop=True)
            nc_.scalar.copy(out=av_sb[:], in_=p_av[:, :H])
            # 1a: S^T = sigmoid(W2^T X^T + Av^T E2 + bc) ; alpha columns
            for c in range(n_chunks):
                c0 = c * CH
                cw = min(CH, Ns - c0)
                pp = psum_a.tile([H, CH], F32, tag="pp")
                nc_.tensor.matmul(pp[:, :cw], lhsT=w2s,
                                  rhs=xT_t[:, c0:c0 + cw],
                                  start=True, stop=False)
                nc_.tensor.matmul(pp[:, :cw], lhsT=av_sb[:],
                                  rhs=e2_sb[:, c0:c0 + cw],
                                  start=False, stop=True)
                s_sb = work.tile([H, CH], BF16, tag="schunk")
                nc_.scalar.activation(s_sb[:, :cw], pp[:, :cw], SIG,
                                      bias=bcs)
                for s in range(cw // H):
                    t = c * (CH // H) + s
                    nc_.tensor.matmul(p_alpha[:, t:t + 1],
                                      lhsT=s_sb[:, s * H:(s + 1) * H],
                                      rhs=qs, start=True, stop=True)
            # alpha = (S^T)^T q + bq, one column per node tile
            nc_.vector.tensor_scalar_add(alpha_sb[:], p_alpha,
                                         float(bq_val))
            # 1c: s_g^T = sum_t (X_t * alpha_t)^T E2T_t
            for t in range(NT):
                xa = work.tile([H, H], BF16, tag="xa")
                nc_.vector.tensor_scalar_mul(xa[:], xnm_t[:, t, :],
                                             alpha_sb[:, t:t + 1])
                nc_.tensor.matmul(p_sg, lhsT=xa[:], rhs=e2t_sb[:, t, :],
                                  start=(t == 0), stop=(t == NT - 1))
            nc_.vector.tensor_copy(out=sg_sb[:], in_=p_sg)
            # s_h^T block: W3a^T v_n + W3b^T s_g + b3
            p_sh = psum_a.tile([H, CH], F32, tag="pp")
            nc_.tensor.matmul(p_sh[:, :Bs], lhsT=w3as, rhs=vn,
                              start=True, stop=False)
            nc_.tensor.matmul(p_sh[:, :Bs], lhsT=w3bs, rhs=sg_sb[:],
                              start=False, stop=True)
            nc_.scalar.activation(sh_out, p_sh[:, :Bs], IDN, bias=b3s)

        # own shard -> s_h block 0, exchanged via the collective
        phase1(xT_sb, xnm_sb, vno_sb[:, :Bs], shT_sb[:, :Bs])
        nc_.scalar.dma_start(out=cc_in[:], in_=shT_sb[:, :Bs])
        nc_.gpsimd.collective_compute(
            "AllGather", mybir.AluOpType.bypass,
            replica_groups=[list(range(M))],
            ins=[cc_in.opt()], outs=[cc_out.opt()])

        eng_i = 0

        # z writes: 4 pieces per block, each issued right after the drains
        # covering its columns so the store stream flows during the block
        # (a short final piece keeps the end-of-kernel flush small).
        PB = [0, 7 * ZCH, 14 * ZCH, 21 * ZCH, Vs]

        def z_block(bci):
            nonlocal eng_i
            lhs = shT_sb[:, bci * H:(bci + 1) * H]
            zt = zout.tile([H, Vs], BF16, tag="zt")
            wi = 0
            for u in range(0, Vs, ZCH):
                uw = min(ZCH, Vs - u)
                zp = psum_z.tile([H, ZCH], F32, tag="zp")
                nc_.tensor.matmul(zp[:, :uw], lhsT=lhs,
                                  rhs=itemT_sb[:, u:u + uw],
                                  start=True, stop=True)
                if eng_i % 9 < 5:
                    nc_.vector.tensor_copy(out=zt[:, u:u + uw],
                                           in_=zp[:, :uw])
                else:
                    nc_.scalar.copy(out=zt[:, u:u + uw], in_=zp[:, :uw])
                eng_i += 1
                done = u + uw
                if done >= PB[wi + 1]:
                    ring = nc_.sync if (bci + wi) % 2 == 0 else nc_.scalar
                    ring.dma_start(
                        out=d_z[bci * H:(bci + 1) * H, PB[wi]:done],
                        in_=zt[:, PB[wi]:done])
                    wi += 1

        # block 0 (own) streams during the collective window; the neighbor
        # shard's phase 1 then recomputes s_h block 1 locally so only
        # blocks 2-7 wait on the gather.
        z_block(0)
        phase1(xT2_sb, xnm2_sb, vno_sb[:, Bs:2 * Bs], shT_sb[:, H:2 * H])
        z_block(1)

        # gathered s_h blocks land straight in shT_sb at rank-rotated
        # offsets; the gpsimd (SWDGE) queue drains them in consumption
        # order without occupying the HWDGE store rings.
        rank_g = nc_.gpsimd.partition_id()
        for j in range(2, M):
            src0 = ((rank_g + j) % M) * H
            nc_.gpsimd.dma_start(out=shT_sb[:, j * H:(j + 1) * H],
                                 in_=cc_out[bass.ds(src0, H), :])

        for bci in range(2, M):
            z_block(bci)

    nc.compile()
    return nc


_CACHE = {}


def _get_program(bq_val):
    key = round(float(bq_val), 10)
    if key not in _CACHE:
        _CACHE[key] = _build_program(bq_val)
    return _CACHE[key]


def kernel(session_embedding, item_emb, batch, num_graphs,
           W1, b1, W2, b2, q, bq, W3, b3):
    import ml_dtypes
    BF = ml_dtypes.bfloat16

    session = np.ascontiguousarray(np.asarray(session_embedding, np.float32))
    item = np.ascontiguousarray(np.asarray(item_emb, np.float32))
    batch = np.asarray(batch)
    W1 = np.asarray(W1, np.float32)
    b1 = np.asarray(b1, np.float32)
    W2 = np.asarray(W2, np.float32)
    b2 = np.asarray(b2, np.float32)
    q = np.asarray(q, np.float32)
    bq = np.asarray(bq, np.float32)
    W3 = np.asarray(W3, np.float32)
    b3 = np.asarray(b3, np.float32)

    uniform = (session.shape == (N, H) and item.shape == (V, H)
               and batch.shape == (N,)
               and int(num_graphs) == B
               and np.array_equal(batch, np.repeat(np.arange(B), L)))
    if not uniform:
        return _kernel_numpy(session, item, batch, W1, b1, W2, b2,
                             q, bq, W3, b3)

    from concourse.bass_utils import run_bass_kernel_spmd

    nc = _get_program(bq[0])

    # ---- host-side shard prep (index bookkeeping + bf16 casts) ----
    last_idx = np.arange(B) * L + (L - 1)
    v_n = session[last_idx]                       # [B, H]
    vnfT = np.ascontiguousarray(v_n.T.astype(BF))  # [H, B]

    gidx = (np.arange(Ns) // L).astype(np.int64)
    E2 = np.zeros((Bs, Ns), BF)
    E2[gidx, np.arange(Ns)] = 1.0
    E2T_t = np.ascontiguousarray(
        E2.T.reshape(NT, H, Bs).transpose(1, 0, 2))  # [H, NT, Bs]

    itemT = np.ascontiguousarray(item.T.astype(BF))  # [H, V]
    sessT = session.T.astype(BF)                     # [H, N]

    wpk = np.empty((H, 4 * H + 1), BF)
    wpk[:, 0 * H:1 * H] = W1.astype(BF)
    wpk[:, 1 * H:2 * H] = W2.astype(BF)
    wpk[:, 2 * H:3 * H] = W3[:H].astype(BF)
    wpk[:, 3 * H:4 * H] = W3[H:].astype(BF)
    wpk[:, 4 * H] = q.reshape(H).astype(BF)
    bpk = np.empty((H, 2), np.float32)
    bpk[:, 0] = b1 + b2
    bpk[:, 1] = b3
    bpk_bits = bpk.view(np.uint16).view(BF)          # [H, 4] raw bf16 view

    xnm_all = [np.ascontiguousarray(
        session[k * Ns:(k + 1) * Ns].astype(BF).reshape(NT, H, H)
        .transpose(1, 0, 2)) for k in range(M)]     # [H, NT, H] per shard

    in_maps = []
    for k in range(M):
        k2 = (k + 1) % M
        vno = np.empty((H, 2 * Bs + 4), BF)
        vno[:, :Bs] = vnfT[:, k * Bs:(k + 1) * Bs]
        vno[:, Bs:2 * Bs] = vnfT[:, k2 * Bs:(k2 + 1) * Bs]
        vno[:, 2 * Bs:] = bpk_bits
        in_maps.append({
            "xT": np.ascontiguousarray(sessT[:, k * Ns:(k + 1) * Ns]),
            "e2": E2,
            "xnm": xnm_all[k],
            "e2t": E2T_t,
            "xT2": np.ascontiguousarray(sessT[:, k2 * Ns:(k2 + 1) * Ns]),
            "xnm2": xnm_all[k2],
            "vno": vno,
            "itemT": np.ascontiguousarray(itemT[:, k * Vs:(k + 1) * Vs]),
            "wpk": wpk,
        })

    res = run_bass_kernel_spmd(nc, in_maps, list(range(M)))

    # un-rotate: core k's local z row-block j holds graphs ((k+j)%M)*Bs..
    z = np.empty((B, V), np.float32)
    for k in range(M):
        zk = np.asarray(res.results[k]["z"]).astype(np.float32)
        for j in range(M):
            gblk = (k + j) % M
            z[gblk * Bs:(gblk + 1) * Bs, k * Vs:(k + 1) * Vs] = \
                zk[j * Bs:(j + 1) * Bs]
    return z


# revision 24
# speedup vs baseline: 1.0753x; 1.0098x over previous
"""Trainium2 Bass kernel for nn_Embedding2Score (session-graph attention +
vocab-scored readout).

Sharding (8 NeuronCores):
  - phase 1 (attention + segment pooling): data-parallel over sessions —
    core k owns graphs [k*128, (k+1)*128) == nodes [k*6400, (k+1)*6400).
  - each core folds its pooled s_g into its own s_h^T block (tiny matmul),
    then one AllGather exchanges the 8 s_h^T blocks (32KB/core). Gathered
    blocks are DMA'd straight into the s_h^T tile — they are exactly the
    lhsT operands for phase 2, so remote blocks need zero post-collective
    compute besides the z matmuls themselves.
  - phase 2 (z = s_h @ item_emb.T): tensor-parallel over the vocab V —
    core k owns item columns [k*12500, (k+1)*12500) and emits z[:, shard].

All matrices are kept in "transposed" (feature-on-partition) layout on
device so every matmul uses the natural [in,out] weight storage as lhsT
with zero on-device transposes. Segment broadcast (v_n -> nodes) and
segment sum are matmuls against 0/1 selector matrices E2 ([graph, node])
and its transpose — constants for the uniform L=50 layout.

Matmul operands are bf16 (PSUM accumulation stays f32). z is written to
DRAM as bf16 and cast to f32 on the host: the store stream is the
roofline term (B*V elements), so halving it halves phase-2 wall time,
and bf16 rounding (~2^-9 relative) is far inside the accuracy budget.

Latency hiding: the collectives runtime pays a one-time bootstrap
barrier that absorbs the inter-core launch skew; each core computes its
OWN graph block's z first (purely local) to overlap that window, and
the remaining 7 blocks' s_h arrive via rank-rotated (partition-id
offset) reads of the gathered buffer; the host un-rotates the z row
blocks. Loads are few, large, and ordered critical-first across both
HWDGE rings because each ring executes its DMAs serially.
"""

from contextlib import ExitStack

import numpy as np

H = 128
B = 1024
L = 50
N = B * L
V = 100000
M = 8            # cores
Bs = B // M      # 128 graphs / core
Ns = N // M      # 6400 nodes / core
Vs = V // M      # 12500 vocab cols / core
NT = Ns // H     # 50 node tiles / core
CH = 512         # phase-1a chunk width (nodes)
ZCH = 512        # phase-2 psum chunk width (vocab cols, 1 PSUM bank)


def _sigmoid(x):
    out = np.empty_like(x)
    np.negative(x, out=out)
    np.exp(out, out=out)
    out += 1.0
    np.reciprocal(out, out=out)
    return out


def _kernel_numpy(session, item, batch, W1, b1, W2, b2, q, bq, W3, b3):
    """General-batch fallback (host only). Handles any sorted batch."""
    nb = int(batch.max()) + 1
    last_idx = np.searchsorted(batch, np.arange(nb), side="right") - 1
    v_n = session[last_idx]
    pre = _sigmoid(v_n[batch] @ W1 + b1 + session @ W2 + b2)
    alpha = pre @ q + bq
    w = alpha * session
    s_g = np.zeros((nb, session.shape[1]), np.float32)
    np.add.at(s_g, batch, w)
    s_h = np.concatenate([v_n, s_g], axis=1) @ W3 + b3
    return (s_h @ item.T).astype(np.float32)


def _build_program(bq_val):
    import concourse.bass as bass
    import concourse.bacc as bacc
    import concourse.tile as tile
    from concourse import mybir

    F32 = mybir.dt.float32
    BF16 = mybir.dt.bfloat16
    SIG = mybir.ActivationFunctionType.Sigmoid
    IDN = mybir.ActivationFunctionType.Identity

    nc = bacc.Bacc("TRN2", target_bir_lowering=False, debug=False,
                   num_devices=M)

    # ---- DRAM I/O (per-core data; identical program on all cores) ----
    d_xT = nc.dram_tensor("xT", [H, Ns], BF16, kind="ExternalInput").ap()
    d_e2 = nc.dram_tensor("e2", [Bs, Ns], BF16, kind="ExternalInput").ap()
    d_xnm = nc.dram_tensor("xnm", [H, NT, H], BF16, kind="ExternalInput").ap()
    d_e2t = nc.dram_tensor("e2t", [H, NT, Bs], BF16,
                           kind="ExternalInput").ap()
    # neighbor shard (rank+1)%M node data: its phase 1 is recomputed
    # locally during the collective window so block 1 needs no gather.
    d_xT2 = nc.dram_tensor("xT2", [H, Ns], BF16, kind="ExternalInput").ap()
    d_xnm2 = nc.dram_tensor("xnm2", [H, NT, H], BF16,
                            kind="ExternalInput").ap()
    # own + neighbor v_n^T [H, 2*Bs] ++ packed biases (bc | b3 as f32,
    # bitcast into 4 bf16 columns) so the sync ring needs one DMA for all.
    d_vno = nc.dram_tensor("vno", [H, 2 * Bs + 4], BF16,
                           kind="ExternalInput").ap()
    d_item = nc.dram_tensor("itemT", [H, Vs], BF16, kind="ExternalInput").ap()
    # packed weights: w1 | w2 | w3a | w3b | q  (columns), bf16
    d_wpk = nc.dram_tensor("wpk", [H, 4 * H + 1], BF16,
                           kind="ExternalInput").ap()
    # z rows are in LOCAL block order; the host maps local block j to
    # global graph block (rank+j)%M when assembling the full output.
    d_z = nc.dram_tensor("z", [B, Vs], BF16, kind="ExternalOutput").ap()

    cc_in = nc.dram_tensor("cc_in", [H, Bs], BF16).ap()
    cc_out = nc.dram_tensor("cc_out", [M * H, Bs], BF16,
                            addr_space="Shared").ap()

    with tile.TileContext(nc) as tc, ExitStack() as ctx:
        nc_ = tc.nc

        consts = ctx.enter_context(tc.tile_pool(name="consts", bufs=1))
        small = ctx.enter_context(tc.tile_pool(name="small", bufs=1))
        item_pool = ctx.enter_context(tc.tile_pool(name="itemp", bufs=1))
        work = ctx.enter_context(tc.tile_pool(name="work", bufs=3))
        big1 = ctx.enter_context(tc.tile_pool(name="big1", bufs=1))
        zout = ctx.enter_context(tc.tile_pool(name="zout", bufs=2))
        psum_a = ctx.enter_context(
            tc.tile_pool(name="psum_a", bufs=2, space="PSUM"))
        # p_alpha and p_sg share one bank (phases 1a/1c are sequential)
        psum_s = ctx.enter_context(
            tc.tile_pool(name="psum_s", bufs=1, space="PSUM"))
        psum_z = ctx.enter_context(
            tc.tile_pool(name="psum_z", bufs=5, space="PSUM"))

        # ---- inputs: 8 large DMAs, critical-first, split over the two
        # HWDGE rings (each ring runs its DMAs serially).
        wpk_sb = consts.tile([H, 4 * H + 1], BF16)
        vno_sb = consts.tile([H, 2 * Bs + 4], BF16)
        itemT_sb = item_pool.tile([H, Vs], BF16)
        xT_sb = big1.tile([H, Ns], BF16)
        e2_sb = big1.tile([Bs, Ns], BF16)
        xnm_sb = big1.tile([H, NT, H], BF16)
        e2t_sb = big1.tile([H, NT, Bs], BF16)
        xT2_sb = big1.tile([H, Ns], BF16)
        xnm2_sb = big1.tile([H, NT, H], BF16)

        vh = Vs // 2
        # scalar (ACT HWDGE) ring
        nc_.scalar.dma_start(out=wpk_sb[:], in_=d_wpk[:])
        nc_.scalar.dma_start(out=xT_sb[:], in_=d_xT[:])
        nc_.scalar.dma_start(out=xnm_sb[:], in_=d_xnm[:])
        nc_.scalar.dma_start(out=xT2_sb[:], in_=d_xT2[:])
        nc_.scalar.dma_start(out=itemT_sb[:, :vh], in_=d_item[:, :vh])
        # sync (SP HWDGE) ring
        nc_.sync.dma_start(out=vno_sb[:], in_=d_vno[:])
        nc_.sync.dma_start(out=e2_sb[:], in_=d_e2[:])
        nc_.sync.dma_start(out=e2t_sb[:], in_=d_e2t[:])
        nc_.sync.dma_start(out=xnm2_sb[:], in_=d_xnm2[:])
        nc_.sync.dma_start(out=itemT_sb[:, vh:], in_=d_item[:, vh:])

        w1s = wpk_sb[:, 0 * H:1 * H]
        w2s = wpk_sb[:, 1 * H:2 * H]
        w3as = wpk_sb[:, 2 * H:3 * H]
        w3bs = wpk_sb[:, 3 * H:4 * H]
        qs = wpk_sb[:, 4 * H:4 * H + 1]
        bpk = vno_sb[:, 2 * Bs:2 * Bs + 4].bitcast(F32)
        bcs = bpk[:, 0:1]
        b3s = bpk[:, 1:2]

        # misc phase-1 results that outlive their producers
        av_sb = small.tile([H, H], BF16)       # (v_n @ W1), graph-major
        alpha_sb = small.tile([H, NT], F32)    # node-tile columns of alpha
        sg_sb = small.tile([H, Bs], BF16)      # s_g^T local shard
        shT_sb = small.tile([H, B], BF16)      # s_h^T, local block order

        p_small = psum_s.tile([H, 192], F32)
        p_alpha = p_small[:, 0:NT]
        p_sg = p_small[:, 64:64 + Bs]
        n_chunks = (Ns + CH - 1) // CH

        def phase1(xT_t, xnm_t, vn, sh_out):
            """attention + segment pooling + s_h fold for one graph shard"""
            # Av = v_n_shard @ W1   -> [graph, h_out]
            p_av = psum_a.tile([H, CH], F32, tag="pp")
            nc_.tensor.matmul(p_av[:, :H], lhsT=vn, rhs=w1s,
                              start=True, stop=True)
            nc_.scalar.copy(out=av_sb[:], in_=p_av[:, :H])
            # 1a: S^T = sigmoid(W2^T X^T + Av^T E2 + bc) ; alpha columns
            for c in range(n_chunks):
                c0 = c * CH
                cw = min(CH, Ns - c0)
                pp = psum_a.tile([H, CH], F32, tag="pp")
                nc_.tensor.matmul(pp[:, :cw], lhsT=w2s,
                                  rhs=xT_t[:, c0:c0 + cw],
                                  start=True, stop=False)
                nc_.tensor.matmul(pp[:, :cw], lhsT=av_sb[:],
                                  rhs=e2_sb[:, c0:c0 + cw],
                                  start=False, stop=True)
                s_sb = work.tile([H, CH], BF16, tag="schunk")
                nc_.scalar.activation(s_sb[:, :cw], pp[:, :cw], SIG,
                                      bias=bcs)
                for s in range(cw // H):
                    t = c * (CH // H) + s
                    nc_.tensor.matmul(p_alpha[:, t:t + 1],
                                      lhsT=s_sb[:, s * H:(s + 1) * H],
                                      rhs=qs, start=True, stop=True)
            # alpha = (S^T)^T q + bq, one column per node tile
            nc_.vector.tensor_scalar_add(alpha_sb[:], p_alpha,
                                         float(bq_val))
            # 1c: s_g^T = sum_t (X_t * alpha_t)^T E2T_t
            for t in range(NT):
                xa = work.tile([H, H], BF16, tag="xa")
                nc_.vector.tensor_scalar_mul(xa[:], xnm_t[:, t, :],
                                             alpha_sb[:, t:t + 1])
                nc_.tensor.matmul(p_sg, lhsT=xa[:], rhs=e2t_sb[:, t, :],
                                  start=(t == 0), stop=(t == NT - 1))
            nc_.vector.tensor_copy(out=sg_sb[:], in_=p_sg)
            # s_h^T block: W3a^T v_n + W3b^T s_g + b3
            p_sh = psum_a.tile([H, CH], F32, tag="pp")
            nc_.tensor.matmul(p_sh[:, :Bs], lhsT=w3as, rhs=vn,
                              start=True, stop=False)
            nc_.tensor.matmul(p_sh[:, :Bs], lhsT=w3bs, rhs=sg_sb[:],
                              start=False, stop=True)
            nc_.scalar.activation(sh_out, p_sh[:, :Bs], IDN, bias=b3s)

        # own shard -> s_h block 0, exchanged via the collective. The
        # bounce rides the gpsimd SWDGE queue: both HWDGE rings still
        # carry input loads at this point and would delay the trigger.
        phase1(xT_sb, xnm_sb, vno_sb[:, :Bs], shT_sb[:, :Bs])
        nc_.gpsimd.dma_start(out=cc_in[:], in_=shT_sb[:, :Bs])
        nc_.gpsimd.collective_compute(
            "AllGather", mybir.AluOpType.bypass,
            replica_groups=[list(range(M))],
            ins=[cc_in.opt()], outs=[cc_out.opt()])

        eng_i = 0

        # z writes: 4 pieces per block, each issued right after the drains
        # covering its columns so the store stream flows during the block
        # (a short final piece keeps the end-of-kernel flush small).
        PB = [0, 7 * ZCH, 14 * ZCH, 21 * ZCH, Vs]

        def z_block(bci):
            nonlocal eng_i
            lhs = shT_sb[:, bci * H:(bci + 1) * H]
            zt = zout.tile([H, Vs], BF16, tag="zt")
            wi = 0
            for u in range(0, Vs, ZCH):
                uw = min(ZCH, Vs - u)
                zp = psum_z.tile([H, ZCH], F32, tag="zp")
                nc_.tensor.matmul(zp[:, :uw], lhsT=lhs,
                                  rhs=itemT_sb[:, u:u + uw],
                                  start=True, stop=True)
                if eng_i % 9 < 5:
                    nc_.vector.tensor_copy(out=zt[:, u:u + uw],
                                           in_=zp[:, :uw])
                else:
                    nc_.scalar.copy(out=zt[:, u:u + uw], in_=zp[:, :uw])
                eng_i += 1
                done = u + uw
                if done >= PB[wi + 1]:
                    ring = nc_.sync if (bci + wi) % 2 == 0 else nc_.scalar
                    ring.dma_start(
                        out=d_z[bci * H:(bci + 1) * H, PB[wi]:done],
                        in_=zt[:, PB[wi]:done])
                    wi += 1

        # block 0 (own) streams during the collective window; the neighbor
        # shard's phase 1 then recomputes s_h block 1 locally so only
        # blocks 2-7 wait on the gather.
        z_block(0)
        phase1(xT2_sb, xnm2_sb, vno_sb[:, Bs:2 * Bs], shT_sb[:, H:2 * H])
        z_block(1)

        # gathered s_h blocks land straight in shT_sb at rank-rotated
        # offsets; the gpsimd (SWDGE) queue drains them in consumption
        # order without occupying the HWDGE store rings.
        rank_g = nc_.gpsimd.partition_id()
        for j in range(2, M):
            src0 = ((rank_g + j) % M) * H
            nc_.gpsimd.dma_start(out=shT_sb[:, j * H:(j + 1) * H],
                                 in_=cc_out[bass.ds(src0, H), :])

        for bci in range(2, M):
            z_block(bci)

    nc.compile()
    return nc


_CACHE = {}


def _get_program(bq_val):
    key = round(float(bq_val), 10)
    if key not in _CACHE:
        _CACHE[key] = _build_program(bq_val)
    return _CACHE[key]


def kernel(session_embedding, item_emb, batch, num_graphs,
           W1, b1, W2, b2, q, bq, W3, b3):
    import ml_dtypes
    BF = ml_dtypes.bfloat16

    session = np.ascontiguousarray(np.asarray(session_embedding, np.float32))
    item = np.ascontiguousarray(np.asarray(item_emb, np.float32))
    batch = np.asarray(batch)
    W1 = np.asarray(W1, np.float32)
    b1 = np.asarray(b1, np.float32)
    W2 = np.asarray(W2, np.float32)
    b2 = np.asarray(b2, np.float32)
    q = np.asarray(q, np.float32)
    bq = np.asarray(bq, np.float32)
    W3 = np.asarray(W3, np.float32)
    b3 = np.asarray(b3, np.float32)

    uniform = (session.shape == (N, H) and item.shape == (V, H)
               and batch.shape == (N,)
               and int(num_graphs) == B
               and np.array_equal(batch, np.repeat(np.arange(B), L)))
    if not uniform:
        return _kernel_numpy(session, item, batch, W1, b1, W2, b2,
                             q, bq, W3, b3)

    from concourse.bass_utils import run_bass_kernel_spmd

    nc = _get_program(bq[0])

    # ---- host-side shard prep (index bookkeeping + bf16 casts) ----
    last_idx = np.arange(B) * L + (L - 1)
    v_n = session[last_idx]                       # [B, H]
    vnfT = np.ascontiguousarray(v_n.T.astype(BF))  # [H, B]

    gidx = (np.arange(Ns) // L).astype(np.int64)
    E2 = np.zeros((Bs, Ns), BF)
    E2[gidx, np.arange(Ns)] = 1.0
    E2T_t = np.ascontiguousarray(
        E2.T.reshape(NT, H, Bs).transpose(1, 0, 2))  # [H, NT, Bs]

    itemT = np.ascontiguousarray(item.T.astype(BF))  # [H, V]
    sessT = session.T.astype(BF)                     # [H, N]

    wpk = np.empty((H, 4 * H + 1), BF)
    wpk[:, 0 * H:1 * H] = W1.astype(BF)
    wpk[:, 1 * H:2 * H] = W2.astype(BF)
    wpk[:, 2 * H:3 * H] = W3[:H].astype(BF)
    wpk[:, 3 * H:4 * H] = W3[H:].astype(BF)
    wpk[:, 4 * H] = q.reshape(H).astype(BF)
    bpk = np.empty((H, 2), np.float32)
    bpk[:, 0] = b1 + b2
    bpk[:, 1] = b3
    bpk_bits = bpk.view(np.uint16).view(BF)          # [H, 4] raw bf16 view

    xnm_all = [np.ascontiguousarray(
        session[k * Ns:(k + 1) * Ns].astype(BF).reshape(NT, H, H)
        .transpose(1, 0, 2)) for k in range(M)]     # [H, NT, H] per shard

    in_maps = []
    for k in range(M):
        k2 = (k + 1) % M
        vno = np.empty((H, 2 * Bs + 4), BF)
        vno[:, :Bs] = vnfT[:, k * Bs:(k + 1) * Bs]
        vno[:, Bs:2 * Bs] = vnfT[:, k2 * Bs:(k2 + 1) * Bs]
        vno[:, 2 * Bs:] = bpk_bits
        in_maps.append({
            "xT": np.ascontiguousarray(sessT[:, k * Ns:(k + 1) * Ns]),
            "e2": E2,
            "xnm": xnm_all[k],
            "e2t": E2T_t,
            "xT2": np.ascontiguousarray(sessT[:, k2 * Ns:(k2 + 1) * Ns]),
            "xnm2": xnm_all[k2],
            "vno": vno,
            "itemT": np.ascontiguousarray(itemT[:, k * Vs:(k + 1) * Vs]),
            "wpk": wpk,
        })

    res = run_bass_kernel_spmd(nc, in_maps, list(range(M)))

    # un-rotate: core k's local z row-block j holds graphs ((k+j)%M)*Bs..
    z = np.empty((B, V), np.float32)
    for k in range(M):
        zk = np.asarray(res.results[k]["z"]).astype(np.float32)
        for j in range(M):
            gblk = (k + j) % M
            z[gblk * Bs:(gblk + 1) * Bs, k * Vs:(k + 1) * Vs] = \
                zk[j * Bs:(j + 1) * Bs]
    return z


# revision 28
# speedup vs baseline: 1.1073x; 1.0297x over previous
"""Trainium2 Bass kernel for nn_Embedding2Score (session-graph attention +
vocab-scored readout).

Sharding (8 NeuronCores):
  - phase 1 (attention + segment pooling): data-parallel over sessions —
    core k owns graphs [k*128, (k+1)*128) == nodes [k*6400, (k+1)*6400).
  - each core folds its pooled s_g into its own s_h^T block (tiny matmul),
    then one AllGather exchanges the 8 s_h^T blocks (32KB/core). Gathered
    blocks are DMA'd straight into the s_h^T tile — they are exactly the
    lhsT operands for phase 2, so remote blocks need zero post-collective
    compute besides the z matmuls themselves.
  - phase 2 (z = s_h @ item_emb.T): tensor-parallel over the vocab V —
    core k owns item columns [k*12500, (k+1)*12500) and emits z[:, shard].

All matrices are kept in "transposed" (feature-on-partition) layout on
device so every matmul uses the natural [in,out] weight storage as lhsT
with zero on-device transposes. Segment broadcast (v_n -> nodes) and
segment sum are matmuls against 0/1 selector matrices E2 ([graph, node])
and its transpose — constants for the uniform L=50 layout.

Matmul operands are bf16 (PSUM accumulation stays f32). z is written to
DRAM as bf16 and cast to f32 on the host: the store stream is the
roofline term (B*V elements), so halving it halves phase-2 wall time,
and bf16 rounding (~2^-9 relative) is far inside the accuracy budget.

Latency hiding: the collectives runtime pays a one-time bootstrap
barrier that absorbs the inter-core launch skew; each core computes its
OWN graph block's z first (purely local) to overlap that window, and
the remaining 7 blocks' s_h arrive via rank-rotated (partition-id
offset) reads of the gathered buffer; the host un-rotates the z row
blocks. Loads are few, large, and ordered critical-first across both
HWDGE rings because each ring executes its DMAs serially.
"""

from contextlib import ExitStack

import numpy as np

H = 128
B = 1024
L = 50
N = B * L
V = 100000
M = 8            # cores
Bs = B // M      # 128 graphs / core
Ns = N // M      # 6400 nodes / core
Vs = V // M      # 12500 vocab cols / core
NT = Ns // H     # 50 node tiles / core
CH = 512         # phase-1a chunk width (nodes)
ZCH = 512        # phase-2 psum chunk width (vocab cols, 1 PSUM bank)


def _sigmoid(x):
    out = np.empty_like(x)
    np.negative(x, out=out)
    np.exp(out, out=out)
    out += 1.0
    np.reciprocal(out, out=out)
    return out


def _kernel_numpy(session, item, batch, W1, b1, W2, b2, q, bq, W3, b3):
    """General-batch fallback (host only). Handles any sorted batch."""
    nb = int(batch.max()) + 1
    last_idx = np.searchsorted(batch, np.arange(nb), side="right") - 1
    v_n = session[last_idx]
    pre = _sigmoid(v_n[batch] @ W1 + b1 + session @ W2 + b2)
    alpha = pre @ q + bq
    w = alpha * session
    s_g = np.zeros((nb, session.shape[1]), np.float32)
    np.add.at(s_g, batch, w)
    s_h = np.concatenate([v_n, s_g], axis=1) @ W3 + b3
    return (s_h @ item.T).astype(np.float32)


def _build_program(bq_val):
    import concourse.bass as bass
    import concourse.bacc as bacc
    import concourse.tile as tile
    from concourse import mybir

    F32 = mybir.dt.float32
    BF16 = mybir.dt.bfloat16
    SIG = mybir.ActivationFunctionType.Sigmoid
    IDN = mybir.ActivationFunctionType.Identity

    nc = bacc.Bacc("TRN2", target_bir_lowering=False, debug=False,
                   num_devices=M)

    # ---- DRAM I/O (per-core data; identical program on all cores) ----
    d_xT = nc.dram_tensor("xT", [H, Ns], BF16, kind="ExternalInput").ap()
    d_e2 = nc.dram_tensor("e2", [Bs, Ns], BF16, kind="ExternalInput").ap()
    d_xnm = nc.dram_tensor("xnm", [H, NT, H], BF16, kind="ExternalInput").ap()
    d_e2t = nc.dram_tensor("e2t", [H, NT, Bs], BF16,
                           kind="ExternalInput").ap()
    # neighbor shard (rank+1)%M node data: its phase 1 is recomputed
    # locally during the collective window so block 1 needs no gather.
    d_xT2 = nc.dram_tensor("xT2", [H, Ns], BF16, kind="ExternalInput").ap()
    d_xnm2 = nc.dram_tensor("xnm2", [H, NT, H], BF16,
                            kind="ExternalInput").ap()
    # own + neighbor v_n^T [H, 2*Bs] ++ packed biases (bc | b3 as f32,
    # bitcast into 4 bf16 columns) so the sync ring needs one DMA for all.
    d_vno = nc.dram_tensor("vno", [H, 2 * Bs + 4], BF16,
                           kind="ExternalInput").ap()
    d_item = nc.dram_tensor("itemT", [H, Vs], BF16, kind="ExternalInput").ap()
    # packed weights: w1 | w2 | w3a | w3b | q  (columns), bf16
    d_wpk = nc.dram_tensor("wpk", [H, 4 * H + 1], BF16,
                           kind="ExternalInput").ap()
    # z rows are in LOCAL block order; the host maps local block j to
    # global graph block (rank+j)%M when assembling the full output.
    d_z = nc.dram_tensor("z", [B, Vs], BF16, kind="ExternalOutput").ap()

    cc_in = nc.dram_tensor("cc_in", [H, Bs], BF16).ap()
    cc_out = nc.dram_tensor("cc_out", [M * H, Bs], BF16,
                            addr_space="Shared").ap()

    with tile.TileContext(nc) as tc, ExitStack() as ctx:
        nc_ = tc.nc

        consts = ctx.enter_context(tc.tile_pool(name="consts", bufs=1))
        small = ctx.enter_context(tc.tile_pool(name="small", bufs=1))
        item_pool = ctx.enter_context(tc.tile_pool(name="itemp", bufs=1))
        work = ctx.enter_context(tc.tile_pool(name="work", bufs=3))
        big1 = ctx.enter_context(tc.tile_pool(name="big1", bufs=1))
        zout = ctx.enter_context(tc.tile_pool(name="zout", bufs=2))
        psum_a = ctx.enter_context(
            tc.tile_pool(name="psum_a", bufs=2, space="PSUM"))
        # p_alpha and p_sg share one bank (phases 1a/1c are sequential)
        psum_s = ctx.enter_context(
            tc.tile_pool(name="psum_s", bufs=1, space="PSUM"))
        psum_z = ctx.enter_context(
            tc.tile_pool(name="psum_z", bufs=5, space="PSUM"))

        # ---- inputs: 8 large DMAs, critical-first, split over the two
        # HWDGE rings (each ring runs its DMAs serially).
        wpk_sb = consts.tile([H, 4 * H + 1], BF16)
        vno_sb = consts.tile([H, 2 * Bs + 4], BF16)
        itemT_sb = item_pool.tile([H, Vs], BF16)
        xT_sb = big1.tile([H, Ns], BF16)
        e2_sb = big1.tile([Bs, Ns], BF16)
        xnm_sb = big1.tile([H, NT, H], BF16)
        e2t_sb = big1.tile([H, NT, Bs], BF16)
        xT2_sb = big1.tile([H, Ns], BF16)
        xnm2_sb = big1.tile([H, NT, H], BF16)

        vh = Vs // 2
        # scalar (ACT HWDGE) ring
        nc_.scalar.dma_start(out=wpk_sb[:], in_=d_wpk[:])
        nc_.scalar.dma_start(out=xT_sb[:], in_=d_xT[:])
        nc_.scalar.dma_start(out=xnm_sb[:], in_=d_xnm[:])
        nc_.scalar.dma_start(out=xT2_sb[:], in_=d_xT2[:])
        nc_.scalar.dma_start(out=itemT_sb[:, :vh], in_=d_item[:, :vh])
        # sync (SP HWDGE) ring
        nc_.sync.dma_start(out=vno_sb[:], in_=d_vno[:])
        nc_.sync.dma_start(out=e2_sb[:], in_=d_e2[:])
        nc_.sync.dma_start(out=e2t_sb[:], in_=d_e2t[:])
        nc_.sync.dma_start(out=xnm2_sb[:], in_=d_xnm2[:])
        nc_.sync.dma_start(out=itemT_sb[:, vh:], in_=d_item[:, vh:])

        w1s = wpk_sb[:, 0 * H:1 * H]
        w2s = wpk_sb[:, 1 * H:2 * H]
        w3as = wpk_sb[:, 2 * H:3 * H]
        w3bs = wpk_sb[:, 3 * H:4 * H]
        qs = wpk_sb[:, 4 * H:4 * H + 1]
        bpk = vno_sb[:, 2 * Bs:2 * Bs + 4].bitcast(F32)
        bcs = bpk[:, 0:1]
        b3s = bpk[:, 1:2]

        # misc phase-1 results that outlive their producers
        av_sb = small.tile([H, H], BF16)       # (v_n @ W1), graph-major
        alpha_sb = small.tile([H, NT], F32)    # node-tile columns of alpha
        sg_sb = small.tile([H, Bs], BF16)      # s_g^T local shard
        shT_sb = small.tile([H, B], BF16)      # s_h^T, local block order

        p_small = psum_s.tile([H, 192], F32)
        p_alpha = p_small[:, 0:NT]
        p_sg = p_small[:, 64:64 + Bs]
        n_chunks = (Ns + CH - 1) // CH

        def phase1(xT_t, xnm_t, vn, sh_out, pool, ptag):
            """attention + segment pooling + s_h fold for one graph shard.
            The second (redundant neighbor) call draws its PSUM from the
            z pool: the buffer-reuse chain pins its matmuls behind block
            0's drains so the scheduler cannot hoist them into phase 1A
            and head-block the tensor queue on the late xT2 load."""
            # Av = v_n_shard @ W1   -> [graph, h_out]
            p_av = pool.tile([H, CH], F32, tag=ptag)
            nc_.tensor.matmul(p_av[:, :H], lhsT=vn, rhs=w1s,
                              start=True, stop=True)
            nc_.scalar.copy(out=av_sb[:], in_=p_av[:, :H])
            # 1a: S^T = sigmoid(W2^T X^T + Av^T E2 + bc) ; alpha columns
            for c in range(n_chunks):
                c0 = c * CH
                cw = min(CH, Ns - c0)
                pp = pool.tile([H, CH], F32, tag=ptag)
                nc_.tensor.matmul(pp[:, :cw], lhsT=w2s,
                                  rhs=xT_t[:, c0:c0 + cw],
                                  start=True, stop=False)
                nc_.tensor.matmul(pp[:, :cw], lhsT=av_sb[:],
                                  rhs=e2_sb[:, c0:c0 + cw],
                                  start=False, stop=True)
                s_sb = work.tile([H, CH], BF16, tag="schunk")
                nc_.scalar.activation(s_sb[:, :cw], pp[:, :cw], SIG,
                                      bias=bcs)
                for s in range(cw // H):
                    t = c * (CH // H) + s
                    nc_.tensor.matmul(p_alpha[:, t:t + 1],
                                      lhsT=s_sb[:, s * H:(s + 1) * H],
                                      rhs=qs, start=True, stop=True)
            # alpha = (S^T)^T q + bq, one column per node tile
            nc_.vector.tensor_scalar_add(alpha_sb[:], p_alpha,
                                         float(bq_val))
            # 1c: s_g^T = sum_t (X_t * alpha_t)^T E2T_t
            for t in range(NT):
                xa = work.tile([H, H], BF16, tag="xa")
                nc_.vector.tensor_scalar_mul(xa[:], xnm_t[:, t, :],
                                             alpha_sb[:, t:t + 1])
                nc_.tensor.matmul(p_sg, lhsT=xa[:], rhs=e2t_sb[:, t, :],
                                  start=(t == 0), stop=(t == NT - 1))
            nc_.vector.tensor_copy(out=sg_sb[:], in_=p_sg)
            # s_h^T block: W3a^T v_n + W3b^T s_g + b3
            p_sh = pool.tile([H, CH], F32, tag=ptag)
            nc_.tensor.matmul(p_sh[:, :Bs], lhsT=w3as, rhs=vn,
                              start=True, stop=False)
            nc_.tensor.matmul(p_sh[:, :Bs], lhsT=w3bs, rhs=sg_sb[:],
                              start=False, stop=True)
            nc_.scalar.activation(sh_out, p_sh[:, :Bs], IDN, bias=b3s)

        # own shard -> s_h block 0, exchanged via the collective. The
        # bounce rides the gpsimd SWDGE queue: both HWDGE rings still
        # carry input loads at this point and would delay the trigger.
        phase1(xT_sb, xnm_sb, vno_sb[:, :Bs], shT_sb[:, :Bs],
               psum_a, "pp")
        nc_.gpsimd.dma_start(out=cc_in[:], in_=shT_sb[:, :Bs])
        nc_.gpsimd.collective_compute(
            "AllGather", mybir.AluOpType.bypass,
            replica_groups=[list(range(M))],
            ins=[cc_in.opt()], outs=[cc_out.opt()])

        eng_i = 0

        # z writes: 4 pieces per block, each issued right after the drains
        # covering its columns so the store stream flows during the block
        # (a short final piece keeps the end-of-kernel flush small).
        PB = [0, 7 * ZCH, 14 * ZCH, 21 * ZCH, Vs]

        def z_block(bci):
            nonlocal eng_i
            lhs = shT_sb[:, bci * H:(bci + 1) * H]
            zt = zout.tile([H, Vs], BF16, tag="zt")
            wi = 0
            for u in range(0, Vs, ZCH):
                uw = min(ZCH, Vs - u)
                zp = psum_z.tile([H, ZCH], F32, tag="zp")
                nc_.tensor.matmul(zp[:, :uw], lhsT=lhs,
                                  rhs=itemT_sb[:, u:u + uw],
                                  start=True, stop=True)
                if eng_i % 9 < 5:
                    nc_.vector.tensor_copy(out=zt[:, u:u + uw],
                                           in_=zp[:, :uw])
                else:
                    nc_.scalar.copy(out=zt[:, u:u + uw], in_=zp[:, :uw])
                eng_i += 1
                done = u + uw
                if done >= PB[wi + 1]:
                    ring = nc_.sync if (bci + wi) % 2 == 0 else nc_.scalar
                    ring.dma_start(
                        out=d_z[bci * H:(bci + 1) * H, PB[wi]:done],
                        in_=zt[:, PB[wi]:done])
                    wi += 1

        # block 0 (own) streams during the collective window; the neighbor
        # shard's phase 1 then recomputes s_h block 1 locally so only
        # blocks 2-7 wait on the gather.
        z_block(0)
        phase1(xT2_sb, xnm2_sb, vno_sb[:, Bs:2 * Bs], shT_sb[:, H:2 * H],
               psum_z, "zp")
        z_block(1)

        # gathered s_h blocks land straight in shT_sb at rank-rotated
        # offsets; the gpsimd (SWDGE) queue drains them in consumption
        # order without occupying the HWDGE store rings.
        rank_g = nc_.gpsimd.partition_id()
        for j in range(2, M):
            src0 = ((rank_g + j) % M) * H
            nc_.gpsimd.dma_start(out=shT_sb[:, j * H:(j + 1) * H],
                                 in_=cc_out[bass.ds(src0, H), :])

        for bci in range(2, M):
            z_block(bci)

    nc.compile()
    return nc


_CACHE = {}


def _get_program(bq_val):
    key = round(float(bq_val), 10)
    if key not in _CACHE:
        _CACHE[key] = _build_program(bq_val)
    return _CACHE[key]


def kernel(session_embedding, item_emb, batch, num_graphs,
           W1, b1, W2, b2, q, bq, W3, b3):
    import ml_dtypes
    BF = ml_dtypes.bfloat16

    session = np.ascontiguousarray(np.asarray(session_embedding, np.float32))
    item = np.ascontiguousarray(np.asarray(item_emb, np.float32))
    batch = np.asarray(batch)
    W1 = np.asarray(W1, np.float32)
    b1 = np.asarray(b1, np.float32)
    W2 = np.asarray(W2, np.float32)
    b2 = np.asarray(b2, np.float32)
    q = np.asarray(q, np.float32)
    bq = np.asarray(bq, np.float32)
    W3 = np.asarray(W3, np.float32)
    b3 = np.asarray(b3, np.float32)

    uniform = (session.shape == (N, H) and item.shape == (V, H)
               and batch.shape == (N,)
               and int(num_graphs) == B
               and np.array_equal(batch, np.repeat(np.arange(B), L)))
    if not uniform:
        return _kernel_numpy(session, item, batch, W1, b1, W2, b2,
                             q, bq, W3, b3)

    from concourse.bass_utils import run_bass_kernel_spmd

    nc = _get_program(bq[0])

    # ---- host-side shard prep (index bookkeeping + bf16 casts) ----
    last_idx = np.arange(B) * L + (L - 1)
    v_n = session[last_idx]                       # [B, H]
    vnfT = np.ascontiguousarray(v_n.T.astype(BF))  # [H, B]

    gidx = (np.arange(Ns) // L).astype(np.int64)
    E2 = np.zeros((Bs, Ns), BF)
    E2[gidx, np.arange(Ns)] = 1.0
    E2T_t = np.ascontiguousarray(
        E2.T.reshape(NT, H, Bs).transpose(1, 0, 2))  # [H, NT, Bs]

    itemT = np.ascontiguousarray(item.T.astype(BF))  # [H, V]
    sessT = session.T.astype(BF)                     # [H, N]

    wpk = np.empty((H, 4 * H + 1), BF)
    wpk[:, 0 * H:1 * H] = W1.astype(BF)
    wpk[:, 1 * H:2 * H] = W2.astype(BF)
    wpk[:, 2 * H:3 * H] = W3[:H].astype(BF)
    wpk[:, 3 * H:4 * H] = W3[H:].astype(BF)
    wpk[:, 4 * H] = q.reshape(H).astype(BF)
    bpk = np.empty((H, 2), np.float32)
    bpk[:, 0] = b1 + b2
    bpk[:, 1] = b3
    bpk_bits = bpk.view(np.uint16).view(BF)          # [H, 4] raw bf16 view

    xnm_all = [np.ascontiguousarray(
        session[k * Ns:(k + 1) * Ns].astype(BF).reshape(NT, H, H)
        .transpose(1, 0, 2)) for k in range(M)]     # [H, NT, H] per shard

    in_maps = []
    for k in range(M):
        k2 = (k + 1) % M
        vno = np.empty((H, 2 * Bs + 4), BF)
        vno[:, :Bs] = vnfT[:, k * Bs:(k + 1) * Bs]
        vno[:, Bs:2 * Bs] = vnfT[:, k2 * Bs:(k2 + 1) * Bs]
        vno[:, 2 * Bs:] = bpk_bits
        in_maps.append({
            "xT": np.ascontiguousarray(sessT[:, k * Ns:(k + 1) * Ns]),
            "e2": E2,
            "xnm": xnm_all[k],
            "e2t": E2T_t,
            "xT2": np.ascontiguousarray(sessT[:, k2 * Ns:(k2 + 1) * Ns]),
            "xnm2": xnm_all[k2],
            "vno": vno,
            "itemT": np.ascontiguousarray(itemT[:, k * Vs:(k + 1) * Vs]),
            "wpk": wpk,
        })

    res = run_bass_kernel_spmd(nc, in_maps, list(range(M)))

    # un-rotate: core k's local z row-block j holds graphs ((k+j)%M)*Bs..
    z = np.empty((B, V), np.float32)
    for k in range(M):
        zk = np.asarray(res.results[k]["z"]).astype(np.float32)
        for j in range(M):
            gblk = (k + j) % M
            z[gblk * Bs:(gblk + 1) * Bs, k * Vs:(k + 1) * Vs] = \
                zk[j * Bs:(j + 1) * Bs]
    return z
